# revision 28
# baseline (speedup 1.0000x reference)
"""Trainium2 Bass kernel: sliding-window multi-head attention with ALiBi.

Reference computation (B=2, S=4096, E=512, H=8, D=64, window 513):
    q = (inputs_q @ w_q);  k = (inputs_kv @ w_k);  v = (inputs_kv @ w_v)
    att = softmax(q k^T / 8 + alibi, sliding window +-256)
    out = (att v) @ w_o
Sharding: 8 cores = 2 batches x 4 sequence quarters (1024 q rows per core).
Each core gets its kv slice with a 256-row halo (zero-padded at sequence
edges).  All cores run the identical program (SPMD); edge handling is pure
data:
  - zero-padded X_kv makes K=V=0 on out-of-range rows,
  - a host-provided validity column appended to V makes the softmax
    denominator (accumulated by the same AV matmul) skip those rows,
  - the window/ALiBi mask is applied as a multiplicative exp-mask after
    exp(): P = exp(S) * G with G Toeplitz in (kv - q), pre-unrolled by the
    host into the 6-chunk score layout.

Layout: scores are computed transposed (S^T[kv, q]) so the AV matmul needs
no on-chip transposes: lhsT = [V | valid], rhs = P^T gives O^T[d, q] plus
the denominator row.

Schedule: the attention stream is ACT-bound (exp of [128,1536] per head),
so everything else hides under it.  Head pairs (2t, 2t+1) share kt/qt tile
t at partitions 0-63/64-127; their score matmuls are issued interleaved
with tile_position (0,0)/(64,0) so the two K=64 matmuls run concurrently
in separate PE row-groups.  The pair's AV outputs land side by side in one
[65,512] PSUM tile (first matmul's start=True zeroes the whole bank), so
softmax normalization costs one fast reciprocal + one Pool broadcast per
pair.  The attention loop runs pair-major (t = 3..0) with the Q/K
projections for pair t-1 issued inside pair t, so the PE executes them in
attention bubbles.  Everything is bf16 except the f32 PSUM accumulators
and the normalization scalars (DMA halves, DVE ops hit 2x modes, FWL
applies to the stationary matmul operands).  The mask multiply alternates
DVE/Pool to balance; the output projection streams per q-block right
after its last pair and is DMA'd straight out of PSUM.
"""

import os
import sys

if "/opt/trn_rl_repo" not in sys.path:
    sys.path.insert(0, "/opt/trn_rl_repo")

import numpy as np

import concourse.bacc as bacc
import concourse.mybir as mybir
import concourse.tile as tile
from concourse.bass_utils import run_bass_kernel_spmd

# ---------------------------------------------------------------- geometry
B, S, E = 2, 4096, 512
H, D = 8, 64
HD = H * D              # 512
HALF = 256              # window half-width (ATTENTION_WINDOW=512 -> 513 wide)
NCORES = 8
SQ = 4                  # sequence shards per batch
QROWS = S // SQ         # 1024 q rows per core
KVROWS = QROWS + 2 * HALF   # 1536 kv rows per core (with halo)
QB = 4                  # q blocks per core
QBLK = QROWS // QB      # 256 q cols per block
NCH = 6                 # kv chunks per q block
CBLK = 128              # kv chunk rows
SP6 = NCH * QBLK        # 1536: all chunks of a q block side by side

F32 = mybir.dt.float32
F32R = mybir.dt.float32r
BF16 = mybir.dt.bfloat16
PV_DT = BF16            # dtype of exp/mask/P^T/V path
QK_DT = BF16            # dtype of q/k tiles (scores path)

PWORK_BUFS = int(os.environ.get("K_PWORK", "4"))
DMA_SPLIT = os.environ.get("K_DMAQ", "0") == "1"  # split input DMAs SP/ACT
DEBUG_OT = os.environ.get("K_DEBUG_OT", "0") == "1"  # dump ot_sb tiles
RECIP_FAST = os.environ.get("K_RECIP", "exact") == "fast"
GSQRT = os.environ.get("K_GSQRT", "0") == "1"   # derive G_h by squaring
G_DT = F32 if GSQRT else PV_DT  # squaring chain needs f32 (bf16 compounds)

_CACHE = {}


def _build_program(repeats=1):
    """Build + compile the SPMD program (cached per process).

    repeats > 1 re-runs the whole computation that many times (same inputs,
    same outputs) - used only for wall-clock HW timing by difference.
    """
    key = ("nc", repeats)
    if key in _CACHE:
        return _CACHE[key]

    nc = bacc.Bacc("TRN2", target_bir_lowering=False, debug=False,
                   enable_asserts=True)

    xq_d = nc.dram_tensor("xqT", [E, QROWS], BF16, kind="ExternalInput")
    xkv_d = nc.dram_tensor("xkvT", [E, KVROWS], BF16, kind="ExternalInput")
    wq_d = nc.dram_tensor("wq", [E, HD], BF16, kind="ExternalInput")
    wk_d = nc.dram_tensor("wk", [E, HD], BF16, kind="ExternalInput")
    wv_d = nc.dram_tensor("wv", [E, HD], BF16, kind="ExternalInput")
    wo_d = nc.dram_tensor("wo", [HD, E], BF16, kind="ExternalInput")
    g_d = nc.dram_tensor("gmask", [1 if GSQRT else H, 128, SP6], G_DT,
                         kind="ExternalInput")
    val_d = nc.dram_tensor("validc", [128, KVROWS // CBLK], F32,
                           kind="ExternalInput")
    y_d = nc.dram_tensor("y", [QROWS, E], F32, kind="ExternalOutput")
    if DEBUG_OT:
        dbg_d = [nc.dram_tensor(f"dbg_ot{t}", [128, QROWS], F32,
                                kind="ExternalOutput") for t in range(4)]
        dbg_sp = nc.dram_tensor("dbg_sp", [128, SP6], F32,
                                kind="ExternalOutput")
        dbg_pm = nc.dram_tensor("dbg_pm", [128, SP6], F32,
                                kind="ExternalOutput")
        dbg_pm2 = nc.dram_tensor("dbg_pm2", [128, SP6], F32,
                                 kind="ExternalOutput")
        dbg_v = nc.dram_tensor("dbg_v", [128, 65 * H], F32,
                               kind="ExternalOutput")
        dbg_av = nc.dram_tensor("dbg_av", [65, QBLK], F32,
                                kind="ExternalOutput")
        dbg_rec = nc.dram_tensor("dbg_rec", [1, QBLK], F32,
                                 kind="ExternalOutput")
        dbg_bc = nc.dram_tensor("dbg_bc", [64, QBLK], F32,
                                kind="ExternalOutput")

    EXP = mybir.ActivationFunctionType.Exp

    with tile.TileContext(nc) as tc:
        with (
            tc.tile_pool(name="wts", bufs=12) as wts,
            tc.tile_pool(name="xqp", bufs=4) as xqp,
            tc.tile_pool(name="xkvp", bufs=4) as xkvp,
            tc.tile_pool(name="gmp", bufs=8) as gmp,
            tc.tile_pool(name="proj", bufs=1) as proj,
            tc.tile_pool(name="pwork", bufs=PWORK_BUFS) as pwork,
            tc.tile_pool(name="small", bufs=4) as small,
            tc.tile_pool(name="bigp", bufs=2, space="PSUM") as bigp,
            tc.tile_pool(name="otp", bufs=2, space="PSUM") as otp,
        ):
            def in_dma(i, dst, src):
                eng = nc.scalar if (DMA_SPLIT and i % 2 == 1) else nc.sync
                eng.dma_start(dst, src)

            def load4(dram, name, tag, cols=HD, bufs=None):
                ts = []
                for e in range(4):
                    t = wts.tile([128, cols], BF16, tag=tag,
                                 name=f"{name}{e}", bufs=bufs)
                    in_dma(e, t[:], dram.ap()[128 * e:128 * (e + 1), :])
                    ts.append(t)
                return ts

            # ---- persistent activation tiles
            qt_sb = [proj.tile([128, QROWS], QK_DT, tag=f"qt{t}",
                               name=f"qt{t}") for t in range(4)]
            kt_sb = [proj.tile([128, KVROWS], QK_DT, tag=f"kt{t}",
                               name=f"kt{t}") for t in range(4)]
            # V tiles: head h occupies cols [65h, 65h+64), col 65h+64 = valid
            v_sb = [proj.tile([128, 65 * H], PV_DT, tag=f"v{b}", name=f"v{b}")
                    for b in range(KVROWS // CBLK)]
            ot_sb = [proj.tile([128, QROWS], BF16, tag=f"ot{t}",
                               name=f"ot{t}") for t in range(4)]

            # DMA issue order (single SP FIFO) is load order: interleave the
            # weight loads with the activation loads so the Q projection can
            # start after ~1.5 MB instead of the full input stream.
            wq_sb = load4(wq_d, "wq", tag="w")

            def copy_ps(eng, dst, src):
                if eng is nc.scalar:
                    nc.scalar.copy(dst, src)
                else:
                    eng.tensor_copy(dst, src)

            def projq_group(t, n, xq_sb, wq_sb, eng):
                ps = bigp.tile([128, SP6], F32, tag="bigp", name="psq")
                for e in range(4):
                    nc.tensor.matmul(
                        ps[:, :512],
                        lhsT=wq_sb[e][:, 128 * t:128 * (t + 1)],
                        rhs=xq_sb[e][:, 512 * n:512 * (n + 1)],
                        start=(e == 0), stop=(e == 3))
                copy_ps(eng, qt_sb[t][:, 512 * n:512 * (n + 1)],
                        ps[:, :512])

            def projk_group(t, n, xkv_sb, wk_sb, eng):
                ps = bigp.tile([128, SP6], F32, tag="bigp", name="psk")
                for e in range(4):
                    nc.tensor.matmul(
                        ps[:, :512],
                        lhsT=wk_sb[e][:, 128 * t:128 * (t + 1)],
                        rhs=xkv_sb[e][:, 512 * n:512 * (n + 1)],
                        start=(e == 0), stop=(e == 3))
                copy_ps(eng, kt_sb[t][:, 512 * n:512 * (n + 1)],
                        ps[:, :512])

            def projq(t, xq_sb, wq_sb, eng):
                for n in range(QROWS // 512):
                    projq_group(t, n, xq_sb, wq_sb, eng)

            def projk(t, xkv_sb, wk_sb, eng):
                for n in range(KVROWS // 512):
                    projk_group(t, n, xkv_sb, wk_sb, eng)

            for rep in range(repeats):
                xq_sb = []
                for e in range(4):
                    t = xqp.tile([128, QROWS], BF16, tag="xq", name=f"xq{e}")
                    in_dma(e, t[:], xq_d.ap()[128 * e:128 * (e + 1), :])
                    xq_sb.append(t)
                if rep == 0:
                    wk_sb = load4(wk_d, "wk", tag="w")
                xkv_sb = []
                for e in range(4):
                    t = xkvp.tile([128, KVROWS], BF16, tag="xkv",
                                  name=f"xkv{e}")
                    in_dma(e, t[:],
                           xkv_d.ap()[128 * e:128 * (e + 1), :])
                    xkv_sb.append(t)
                if rep == 0:
                    wv_sb = load4(wv_d, "wv", tag="w")
                    valid_sb = small.tile([128, KVROWS // CBLK], F32,
                                          tag="validc", name="validc")
                    nc.sync.dma_start(valid_sb[:], val_d.ap()[:])
                    ones8 = small.tile([128, H], F32, tag="ones8",
                                       name="ones8")
                    nc.vector.memset(ones8[:], 1.0)

                # ---- Toeplitz exp-masks (own pool: DMA streams during the
                # projections; descending h = first-needed first).  With
                # GSQRT only G7 ships; G_h = G_{h+1}^2 (slopes double as h
                # decreases) chains on ACT during the DMA-bound startup.
                g_sb = [None] * H
                for h in range(H - 1, -1, -1):
                    t = gmp.tile([128, SP6], G_DT, tag="gm", name=f"g{h}")
                    if GSQRT and h < H - 1:
                        nc.scalar.square(t[:], g_sb[h + 1][:])
                    else:
                        in_dma(h, t[:], g_d.ap()[0 if GSQRT else h])
                    g_sb[h] = t
                if rep == 0:
                    wo_sb = load4(wo_d, "wo", tag="wo", cols=E, bufs=4)

                def projv(blk):
                    ps = bigp.tile([128, SP6], F32, tag="bigp", name="psv")
                    for e in range(4):
                        nc.tensor.matmul(
                            ps[:, :512],
                            lhsT=xkv_sb[e][:, 128 * blk:128 * (blk + 1)],
                            rhs=wv_sb[e][:],
                            start=(e == 0), stop=(e == 3))
                    vv = v_sb[blk][:].rearrange("p (h c) -> p h c", c=65)
                    nc.scalar.copy(
                        vv[:, :, 0:64],
                        ps[:, :512].rearrange("p (h c) -> p h c", c=64))
                    nc.vector.tensor_scalar_mul(
                        vv[:, :, 64], ones8[:],
                        valid_sb[:, blk:blk + 1])

                # ---- initial projections: pair 3's q/k + the V blocks the
                # first two q blocks need; the rest interleave into pair 3
                projq(3, xq_sb, wq_sb, nc.scalar)
                projk(3, xkv_sb, wk_sb, nc.scalar)
                for blk in range(8):
                    projv(blk)

                # ---- attention, pair-major (pair t = heads 2t+1 / 2t);
                # next pair's projections interleave between q blocks so the
                # 2-slot bigp ring never sees a burst of allocations
                for t in range(3, -1, -1):
                    pending = []
                    if t == 3:
                        pending = [lambda b=b: projv(b) for b in range(8, 12)]
                    if t > 0:
                        pending += (
                            [lambda n=n: projq_group(t - 1, n, xq_sb, wq_sb,
                                                     nc.vector)
                             for n in range(QROWS // 512)] +
                            [lambda n=n: projk_group(t - 1, n, xkv_sb, wk_sb,
                                                     nc.vector)
                             for n in range(KVROWS // 512)])
                    for qb in range(QB):
                        q0 = QBLK * qb
                        sp_hi = bigp.tile([128, SP6], F32, tag="bigp",
                                          name="sphi")
                        sp_lo = bigp.tile([128, SP6], F32, tag="bigp",
                                          name="splo")
                        qs_hi = qt_sb[t][64:128, q0:q0 + QBLK]
                        qs_lo = qt_sb[t][0:64, q0:q0 + QBLK]
                        # interleaved hi/lo issue: distinct PE row groups run
                        # concurrently (tile_position row 64 / 0)
                        for c in range(NCH):
                            k0 = q0 + CBLK * c
                            nc.tensor.matmul(
                                sp_hi[:, QBLK * c:QBLK * (c + 1)],
                                lhsT=kt_sb[t][64:128, k0:k0 + CBLK],
                                rhs=qs_hi,
                                start=(c % 2 == 0), stop=True,
                                skip_group_check=(c % 2 == 1),
                                tile_position=(64, 0))
                            nc.tensor.matmul(
                                sp_lo[:, QBLK * c:QBLK * (c + 1)],
                                lhsT=kt_sb[t][0:64, k0:k0 + CBLK],
                                rhs=qs_lo,
                                start=(c % 2 == 0), stop=True,
                                skip_group_check=(c % 2 == 1),
                                tile_position=(0, 0))

                        if DEBUG_OT and t == 3 and qb == 0 and rep == 0:
                            spst = pwork.tile([128, SP6], F32, tag="dbgs",
                                              name="spst")
                            nc.vector.tensor_copy(spst[:], sp_hi[:])
                            nc.sync.dma_start(dbg_sp.ap()[:], spst[:])
                        # exp + mask for both heads; both muls on DVE in
                        # issue order, so AV-hi (bank-zeroing start) is
                        # always ready, hence scheduled, before AV-lo
                        pms = []
                        for i, (sp, hh) in enumerate(
                                ((sp_hi, 2 * t + 1), (sp_lo, 2 * t))):
                            pe6 = pwork.tile([128, SP6], PV_DT, tag="pw",
                                             name="pe6")
                            nc.scalar.activation(pe6[:], sp[:], EXP)
                            pm6 = pwork.tile([128, SP6], PV_DT, tag="pw",
                                             name="pm6")
                            nc.vector.tensor_mul(pm6[:], pe6[:], g_sb[hh][:])
                            if DEBUG_OT and t == 3 and qb == 0 and rep == 0:
                                pmst = pwork.tile([128, SP6], F32, tag="dbgs",
                                                  name="pmst")
                                nc.vector.tensor_copy(pmst[:], pm6[:])
                                nc.sync.dma_start(
                                    (dbg_pm if i == 0 else dbg_pm2).ap()[:],
                                    pmst[:])
                            pms.append(pm6)
                        # paired AV: both heads' O^T + denominators in one
                        # PSUM bank -> one reciprocal + one broadcast
                        ot2 = otp.tile([65, 2 * QBLK], F32, tag="otp",
                                       name="ot2")
                        for i, hh in ((0, 2 * t + 1), (1, 2 * t)):
                            for c in range(NCH):
                                nc.tensor.matmul(
                                    ot2[:, QBLK * i:QBLK * (i + 1)],
                                    lhsT=v_sb[2 * qb + c][:,
                                                          65 * hh:65 * hh + 65],
                                    rhs=pms[i][:, QBLK * c:QBLK * (c + 1)],
                                    start=(i == 0 and c == 0),
                                    stop=(i == 1 and c == NCH - 1),
                                    skip_group_check=not (
                                        (i == 0 and c == 0)
                                        or (i == 1 and c == NCH - 1)))
                        rec = small.tile([1, 2 * QBLK], F32, tag="rec",
                                         name="rec")
                        nc.vector.reciprocal(rec[:], ot2[64:65, :])
                        bc = pwork.tile([64, 2 * QBLK], F32, tag="bc",
                                        name="bc")
                        nc.gpsimd.partition_broadcast(bc[:], rec[:])
                        if DEBUG_OT and t == 3 and qb == 0 and rep == 0:
                            avst = pwork.tile([65, QBLK], F32, tag="dbga",
                                              name="avst")
                            nc.vector.tensor_copy(avst[:], ot2[:, :QBLK])
                            nc.sync.dma_start(dbg_av.ap()[:], avst[:])
                            nc.sync.dma_start(dbg_rec.ap()[:], rec[:, :QBLK])
                            nc.sync.dma_start(dbg_bc.ap()[:], bc[:, :QBLK])
                        nc.vector.tensor_mul(
                            ot_sb[t][64:128, q0:q0 + QBLK],
                            ot2[0:64, 0:QBLK], bc[:, 0:QBLK])
                        nc.vector.tensor_mul(
                            ot_sb[t][0:64, q0:q0 + QBLK],
                            ot2[0:64, QBLK:2 * QBLK], bc[:, QBLK:2 * QBLK])

                        # ---- output projection per q block after its last
                        # pair (t == 0); 2 slabs of 128 rows share one bigp
                        # tile in separate banks, DMA'd straight from PSUM
                        if t == 0:
                            for j in range(2):
                                yb = 2 * qb + j
                                yp = otp.tile([128, 512], F32, tag="otp",
                                              name="yp")
                                for tt in range(4):
                                    nc.tensor.matmul(
                                        yp[:],
                                        lhsT=ot_sb[tt][:,
                                                       128 * yb:128 * (yb + 1)],
                                        rhs=wo_sb[tt][:],
                                        start=(tt == 0), stop=(tt == 3))
                                ystage = pwork.tile([128, 512], F32,
                                                    tag="ys", name="ystage")
                                nc.vector.tensor_copy(ystage[:], yp[:])
                                nc.sync.dma_start(
                                    y_d.ap()[128 * yb:128 * (yb + 1), :],
                                    ystage[:])

                        npop = (len(pending) + QB - 1 - qb) // (QB - qb)
                        for _ in range(npop):
                            if pending:
                                pending.pop(0)()

                if DEBUG_OT and rep == 0:
                    vst = pwork.tile([128, 65 * H], F32, tag="dbgo",
                                     name="vst")
                    nc.vector.tensor_copy(vst[:], v_sb[2][:])
                    nc.sync.dma_start(dbg_v.ap()[:], vst[:])
                    for t in range(4):
                        st = pwork.tile([128, QROWS], F32, tag="dbgo",
                                        name="st")
                        nc.vector.tensor_copy(st[:], ot_sb[t][:])
                        nc.sync.dma_start(dbg_d[t].ap()[:], st[:])



    nc.compile()
    _CACHE[key] = nc
    return nc


def build_in_maps(inputs_q, inputs_kv, w_q, w_k, w_v, w_o):
    """Host-side sharding: slice/transpose/pad per core + mask tensors."""
    np_bf = mybir.dt.np(BF16)
    np_pv = mybir.dt.np(PV_DT)
    inputs_q = np.asarray(inputs_q, np.float32)
    inputs_kv = np.asarray(inputs_kv, np.float32)

    wq = np.ascontiguousarray(
        (np.asarray(w_q, np.float32) * 0.125).astype(np_bf))
    wk = np.ascontiguousarray(np.asarray(w_k, np.float32).astype(np_bf))
    wv = np.ascontiguousarray(np.asarray(w_v, np.float32).astype(np_bf))
    wo = np.ascontiguousarray(np.asarray(w_o, np.float32).astype(np_bf))

    # Toeplitz exp-mask, pre-unrolled into the 6-chunk score layout:
    # chunk c, kv row r, q col i -> rel = i - r - 128c + 256
    slopes = np.array([2.0 ** (-(i + 1)) for i in range(H)], np.float64)
    r = np.arange(128)[:, None]
    i = np.arange(QBLK)[None, :]
    np_g = mybir.dt.np(G_DT)
    nheads_g = 1 if GSQRT else H
    g = np.empty((nheads_g, 128, SP6), np_g)
    for c in range(NCH):
        rel = i - r - 128 * c + 256
        band = (np.abs(rel) <= HALF)
        for hh in range(nheads_g):
            s = slopes[H - 1] if GSQRT else slopes[hh]
            g[hh, :, QBLK * c:QBLK * (c + 1)] = (
                np.exp(-s * np.abs(rel)) * band).astype(np_g)

    in_maps = []
    for c in range(NCORES):
        b, sq = divmod(c, SQ)
        g0 = QROWS * sq
        xq = np.ascontiguousarray(inputs_q[b, g0:g0 + QROWS, :].T
                                  .astype(np_bf))
        kvlo = g0 - HALF
        lo, hi = max(0, kvlo), min(S, g0 + QROWS + HALF)
        xkv = np.zeros((E, KVROWS), np_bf)
        xkv[:, lo - kvlo:hi - kvlo] = inputs_kv[b, lo:hi, :].T.astype(np_bf)
        valid = np.zeros((KVROWS,), np.float32)
        valid[lo - kvlo:hi - kvlo] = 1.0
        validc = np.ascontiguousarray(valid.reshape(KVROWS // CBLK, CBLK).T)
        in_maps.append({
            "xqT": xq, "xkvT": xkv,
            "wq": wq, "wk": wk, "wv": wv, "wo": wo,
            "gmask": g, "validc": validc,
        })
    return in_maps


def assemble_output(results):
    out = np.empty((B, S, E), np.float32)
    for c in range(NCORES):
        b, sq = divmod(c, SQ)
        out[b, QROWS * sq:QROWS * (sq + 1), :] = results[c]["y"]
    return out


def kernel(inputs_q, inputs_kv, w_q, w_k, w_v, w_o):
    nc = _build_program()
    in_maps = build_in_maps(inputs_q, inputs_kv, w_q, w_k, w_v, w_o)
    res = run_bass_kernel_spmd(nc, in_maps, core_ids=list(range(NCORES)))
    return assemble_output(res.results)
